# revision 10
# baseline (speedup 1.0000x reference)
"""AdaptivePriorBoxesLoss on 8 Trainium2 NeuronCores (Bass/Tile).

Sparse cell-bucketed formulation. Host quantile-bins the P=262144 priors
into 8 y-bands (one per core) x 16 x-cells of exactly 2048 priors. For
each cell only the truths whose boxes can overlap the cell's prior hull
are evaluated (~30 of 128). The (cell, truth) work pairs are packed into
128 partition-row "slots" x NBLK blocks; each row serves one fixed cell
(so its 2048-prior data is loaded once) and visits NBLK of its cell's
truths, one per block, delivered as per-partition scalars.

Per block the device computes, on [128, 2048] bf16 tiles:
    t1=min(px2,tx2)  t2=max(px1,tx1)  w=t1-t2   wr=relu(w)     (x chain)
    u1=min(py2,ty2)  u2=max(py1,ty1)  h=u1-u2   hr=relu(h)     (y chain)
    inter=wr*hr  den0=pa-inter
    lnI=Ln(inter)  lnD=Ln(den0+ta)          (scalar engine, f32)
    iou=lnI-lnD (+ fused per-row max -> MAXC[:,b])
    BTOP=max(BTOP, iou)                      (running best-truth-overlap)
    cand=(iou>=maxc*(1+2^-7)) * (idx-BIG); CID[:,b]=min(cand)  (argmax)

Host combines: per-cell max of BTOP slot rows -> ln(bto) per prior, the
filter/sum reductions, and the <=128 best-prior scatter corrections,
exactly mirroring the reference semantics.
"""

import os
import sys
from contextlib import ExitStack

for _p in ("/opt/trn_rl_repo", os.path.expanduser("~/.axon_site/_ro/trn_rl_repo")):
    if os.path.isdir(_p) and _p not in sys.path:
        sys.path.insert(0, _p)

import numpy as np
import ml_dtypes

import concourse.bass as bass
import concourse.bacc as bacc
import concourse.mybir as mybir
from concourse import tile
from concourse.bass_utils import run_bass_kernel_spmd
from concourse import dve_ops, dve_spec
from concourse.dve_spec import (
    Spec, Src0, Src1, C0, C1, C2, Zero, Idx, relu, minn, maxx, select, lower,
)
from concourse.dve_uop import DveOpSpec

BF16NP = ml_dtypes.bfloat16

P = 262144
T = 128
NCORES = 8
NCELL = 16
CPC = 2048
PPC = NCELL * CPC          # priors per core = 32768
BIG = 1048576.0
BETA = 1.0
K = 2.5
IOU_THRESH = 0.4

F32 = mybir.dt.float32
BF16 = mybir.dt.bfloat16
ALU = mybir.AluOpType
ACTF = mybir.ActivationFunctionType

NEG = -3.0e38
EPS_MUL = 1.0 + 2.0 ** -7   # widen argmax match by one bf16 ulp


def _register_dve_op(name, spec, subdim=False):
    """Register a custom DVE op at runtime (self-contained kernel.py)."""
    for op in dve_ops.OPS:
        if op.name == name:
            return op
    row = dve_ops._CUSTOM_DVE_ROW_BASE + len(dve_ops.OPS)
    assert row < 0x20, "custom-DVE opcode rows exhausted"
    dve_ops._SUB_OPCODE_FOR_NAME[name] = row
    shas = {}
    for ver in ("v3", "v4"):
        s = DveOpSpec(
            name=name, opcode=row, uops=lower(spec, ver=ver),
            rd1_en=dve_spec._has_src1(spec),
        )
        shas[ver] = s.sha(ver)
    op = dve_ops.DveOp(name, spec, subdim, uops_sha=shas)
    dve_ops.OPS.append(op)
    dve_ops.CUSTOM_DVE_SPECS[name] = spec
    return op


def _np_spanw(in0, in1, s0, s1, imm2):
    return np.maximum(
        np.minimum(in0.astype(np.float32), s0)
        - np.maximum(in1.astype(np.float32), s1), 0.0)


def _np_submax(in0, in1, s0, s1, imm2):
    b = in0.astype(np.float32) - in1.astype(np.float32)
    bm = np.where(np.isnan(b), -np.inf, b)
    acc = np.maximum(bm.max(axis=-1, keepdims=True).reshape(b.shape[0], -1)
                     .max(axis=-1, keepdims=True), s1)
    return b, acc


def _np_selmin(in0, in1, s0, s1, imm2):
    idx = np.arange(in0.shape[-1], dtype=np.float32)[None, :]
    b = np.where(in0.astype(np.float32) >= s0, idx, imm2).astype(np.float32)
    b = b.reshape(in0.shape[0], -1)
    acc = np.minimum(b.min(axis=-1, keepdims=True), s1)
    return b, acc


# wr = relu(min(hi, t_hi) - max(lo, t_lo)) — the clipped 1-D span
SPANW_ANT = _register_dve_op(
    "SPANW_ANT",
    Spec(body=relu(minn(Src0, C0) - maxx(Src1, C1)), reference=_np_spanw),
)
# out = in0 - in1; accum_out = max(out) (seeded from C1)
SUBMAX_ANT = _register_dve_op(
    "SUBMAX_ANT",
    Spec(body=Src0 - Src1, accum=maxx, accum_init=C1, reference=_np_submax),
)
# out = (in0 >= c0) ? elem_idx : imm2; accum_out = min(out) (seeded from C1)
SELMIN_ANT = _register_dve_op(
    "SELMIN_ANT",
    Spec(body=select(Src0 >= C0, Idx, C2), accum=minn,
         accum_init=C1, reference=_np_selmin),
)


def build_nc(nblk):
    nc = bacc.Bacc()

    pd_e = nc.declare_dram_parameter("pd", [128, 5 * CPC], BF16, isOutput=False)
    ts_e = nc.declare_dram_parameter("tscal", [128, 8 * nblk], F32, isOutput=False)
    btop_o = nc.declare_dram_parameter("btop_out", [128, CPC], BF16, isOutput=True)
    maxc_o = nc.declare_dram_parameter("maxc_out", [128, nblk], F32, isOutput=True)
    cid_o = nc.declare_dram_parameter("cid_out", [128, nblk], F32, isOutput=True)

    with ExitStack() as es:
        tc = es.enter_context(tile.TileContext(nc))
        cpool = es.enter_context(tc.tile_pool(name="const", bufs=1))
        wpool = es.enter_context(tc.tile_pool(name="work", bufs=3))

        TSCAL = cpool.tile([128, 8 * nblk], F32, tag="TSCAL")
        nc.sync.dma_start(out=TSCAL[:], in_=ts_e[:])

        def pdarr(i, tag):
            t_ = cpool.tile([128, CPC], BF16, tag=tag)
            nc.sync.dma_start(out=t_[:], in_=pd_e[:, i * CPC:(i + 1) * CPC])
            return t_[:]

        PX1 = pdarr(0, "PX1")
        PX2 = pdarr(1, "PX2")
        PY1 = pdarr(2, "PY1")
        PY2 = pdarr(3, "PY2")
        PA = pdarr(4, "PA")

        BTOP = cpool.tile([128, CPC], BF16, tag="BTOP")
        nc.vector.memset(BTOP[:], float("-inf"))
        MAXC = cpool.tile([128, nblk], F32, tag="MAXC")
        CID = cpool.tile([128, nblk], F32, tag="CID")

        for b in range(nblk):
            def sc(j):
                return TSCAL[:, 8 * b + j:8 * b + j + 1]

            wr = wpool.tile([128, CPC], BF16, tag="wr")
            nc.vector._custom_dve(
                SPANW_ANT, out=wr[:], in0=PX2, in1=PX1, s0=sc(0), s1=sc(1)
            )
            hr = wpool.tile([128, CPC], BF16, tag="hr")
            nc.vector._custom_dve(
                SPANW_ANT, out=hr[:], in0=PY2, in1=PY1, s0=sc(2), s1=sc(3)
            )

            inter = wpool.tile([128, CPC], BF16, tag="inter")
            nc.vector.tensor_tensor(inter[:], wr[:], hr[:], ALU.mult)
            den0 = wpool.tile([128, CPC], BF16, tag="den0")
            nc.gpsimd.tensor_tensor(den0[:], PA, inter[:], ALU.subtract)

            lnI = wpool.tile([128, CPC], F32, tag="lnI")
            nc.scalar.activation(lnI[:], inter[:], ACTF.Ln)
            lnD = wpool.tile([128, CPC], F32, tag="lnD")
            nc.scalar.activation(lnD[:], den0[:], ACTF.Ln, bias=sc(4))

            iou = wpool.tile([128, CPC], BF16, tag="iou")
            nc.vector._custom_dve(
                SUBMAX_ANT, out=iou[:], in0=lnI[:], in1=lnD[:],
                s1=NEG, accum_out=MAXC[:, b:b + 1],
            )

            nc.vector.tensor_tensor(BTOP[:], BTOP[:], iou[:], ALU.max)

            mce = wpool.tile([128, 1], F32, tag="mce")
            nc.vector.tensor_scalar_mul(mce[:], MAXC[:, b:b + 1], EPS_MUL)
            cand = wpool.tile([128, CPC], F32, tag="cand")
            nc.vector._custom_dve(
                SELMIN_ANT, out=cand[:], in0=iou[:],
                s0=mce[:], s1=3.0e38, imm2=3.0e38,
                accum_out=CID[:, b:b + 1],
            )

        nc.sync.dma_start(out=btop_o[:], in_=BTOP[:])
        nc.sync.dma_start(out=maxc_o[:], in_=MAXC[:])
        nc.sync.dma_start(out=cid_o[:], in_=CID[:])

    nc.finalize()
    return nc


def _host_prep(locs, params, truths):
    """Bucket priors, build per-core slot schedules and device inputs."""
    px = locs[:, 0]
    py = locs[:, 1]
    pw = params[:, 0]
    ph = params[:, 1]
    px1 = px - pw / 2
    px2 = px + pw / 2
    py1 = py - ph / 2
    py2 = py + ph / 2
    pa = pw * ph
    salpha = 1.0 / (1.0 + np.exp(-params[:, 2].astype(np.float64)))

    order_y = np.argsort(py, kind="stable")
    flat_cells = np.empty((NCORES * NCELL, CPC), dtype=np.int64)
    for c in range(NCORES):
        band = order_y[c * PPC:(c + 1) * PPC]
        band = band[np.argsort(px[band], kind="stable")]
        flat_cells[c * NCELL:(c + 1) * NCELL] = band.reshape(NCELL, CPC)

    tx1, ty1, tx2, ty2 = truths[:, 0], truths[:, 1], truths[:, 2], truths[:, 3]
    ta = (tx2 - tx1) * (ty2 - ty1)

    # per-cell truth lists from exact prior-box hulls
    ncc = NCORES * NCELL
    flat_lists = []
    for cc in range(ncc):
        pp = flat_cells[cc]
        hx1 = px1[pp].min(); hx2 = px2[pp].max()
        hy1 = py1[pp].min(); hy2 = py2[pp].max()
        hit = (tx1 <= hx2) & (tx2 >= hx1) & (ty1 <= hy2) & (ty2 >= hy1)
        flat_lists.append(np.nonzero(hit)[0])

    # pick smallest NBLK for which cells can be bin-packed into cores with
    # <=128 row-slots each (LPT greedy), then apply that assignment
    nblk = None
    for cand in range(1, 17):
        slots = np.array([-(-len(l) // cand) for l in flat_lists])
        order = np.argsort(-slots, kind="stable")
        loads = np.zeros(NCORES, dtype=np.int64)
        counts = np.zeros(NCORES, dtype=np.int64)
        assign = np.full(ncc, -1, dtype=np.int64)
        for cc in order:
            feas = np.nonzero(counts < NCELL)[0]
            tgt = feas[np.argmin(loads[feas])]
            assign[cc] = tgt
            loads[tgt] += slots[cc]
            counts[tgt] += 1
        if loads.max() <= 128:
            nblk = cand
            break
    assert nblk is not None

    perm = np.empty(P, dtype=np.int64)
    pp_all = np.empty((NCORES, NCELL, CPC), dtype=np.int64)
    lists = [[] for _ in range(NCORES)]
    fill = np.zeros(NCORES, dtype=np.int64)
    for cc in range(ncc):
        c = assign[cc]
        g = fill[c]
        fill[c] += 1
        pp_all[c, g] = flat_cells[cc]
        lists[c].append(flat_lists[cc])
        perm[c * PPC + g * CPC:(c * PPC + (g + 1) * CPC)] = flat_cells[cc]

    # slot assignment
    rowcell = np.full((NCORES, 128), -1, dtype=np.int64)
    rowslot = np.zeros((NCORES, 128), dtype=np.int64)
    rowbase = np.zeros((NCORES, NCELL), dtype=np.int64)
    for c in range(NCORES):
        r = 0
        for g in range(NCELL):
            rowbase[c, g] = r
            ns = -(-len(lists[c][g]) // nblk)
            for k in range(ns):
                rowcell[c, r] = g
                rowslot[c, r] = k
                r += 1

    in_maps = []
    for c in range(NCORES):
        pd = np.zeros((128, 5 * CPC), dtype=BF16NP)
        tscal = np.zeros((128, 8 * nblk), dtype=np.float32)
        for r in range(128):
            g = rowcell[c, r]
            if g < 0:
                tscal[r, 0::8] = -9.99
                tscal[r, 1::8] = -10.0
                tscal[r, 2::8] = -9.99
                tscal[r, 3::8] = -10.0
                tscal[r, 4::8] = 1.0
                continue
            pp = pp_all[c, g]
            pd[r, 0 * CPC:1 * CPC] = px1[pp].astype(BF16NP)
            pd[r, 1 * CPC:2 * CPC] = px2[pp].astype(BF16NP)
            pd[r, 2 * CPC:3 * CPC] = py1[pp].astype(BF16NP)
            pd[r, 3 * CPC:4 * CPC] = py2[pp].astype(BF16NP)
            pd[r, 4 * CPC:5 * CPC] = pa[pp].astype(BF16NP)
            lst = lists[c][g]
            k = rowslot[c, r]
            for b in range(nblk):
                pos = k * nblk + b
                if pos < len(lst):
                    t = lst[pos]
                    tscal[r, 8 * b + 0] = tx2[t]
                    tscal[r, 8 * b + 1] = tx1[t]
                    tscal[r, 8 * b + 2] = ty2[t]
                    tscal[r, 8 * b + 3] = ty1[t]
                    tscal[r, 8 * b + 4] = ta[t]
                else:
                    tscal[r, 8 * b + 0] = -9.99
                    tscal[r, 8 * b + 1] = -10.0
                    tscal[r, 8 * b + 2] = -9.99
                    tscal[r, 8 * b + 3] = -10.0
                    tscal[r, 8 * b + 4] = 1.0
        in_maps.append({"pd": pd, "tscal": tscal})

    meta = dict(
        perm=perm, lists=lists, nblk=nblk, rowcell=rowcell,
        rowbase=rowbase, salpha=salpha, pp_all=pp_all,
    )
    return in_maps, meta


def _combine(results, meta):
    perm = meta["perm"]
    lists = meta["lists"]
    nblk = meta["nblk"]
    rowcell = meta["rowcell"]
    rowbase = meta["rowbase"]
    salpha = meta["salpha"]

    ln_thresh = np.log(IOU_THRESH)

    bto = np.full(P, -np.inf, dtype=np.float64)   # permuted order, ln-domain
    maxc = []
    cid = []
    for c in range(NCORES):
        btop = np.asarray(results[c]["btop_out"]).astype(np.float32)
        maxc.append(np.asarray(results[c]["maxc_out"]))
        cid.append(np.asarray(results[c]["cid_out"]))
        for g in range(NCELL):
            rows = np.nonzero(rowcell[c] == g)[0]
            m = btop[rows[0]]
            for r in rows[1:]:
                m = np.maximum(m, btop[r])
            bto[c * PPC + g * CPC:(c * PPC + (g + 1) * CPC)] = m

    salpha_p = salpha[perm]
    F = bto > ln_thresh
    s_alpha = salpha.sum()
    base_num = (salpha_p[F] * bto[F]).sum()
    base_den = float(F.sum())

    # per-truth winner (bpo in ln domain, bpi as permuted global index)
    bpo = np.full(T, -np.inf)
    bpi_perm = np.zeros(T, dtype=np.int64)
    bpi_orig = np.zeros(T, dtype=np.int64)
    for t in range(T):
        best = -np.inf
        best_orig = None
        best_perm = 0
        for c in range(NCORES):
            for g in range(NCELL):
                pos_arr = np.nonzero(lists[c][g] == t)[0]
                if not len(pos_arr):
                    continue
                pos = int(pos_arr[0])
                k, b = divmod(pos, nblk)
                r = rowbase[c, g] + k
                m = float(maxc[c][r, b])
                if m <= NEG:
                    continue
                idx = cid[c][r, b]
                if not (0 <= idx < CPC):
                    idx = 0.0
                gp = c * PPC + g * CPC + int(idx)
                go = int(perm[gp])
                if m > best or (m == best and go < best_orig):
                    best = m
                    best_orig = go
                    best_perm = gp
        bpo[t] = best
        bpi_perm[t] = best_perm
        bpi_orig[t] = best_orig if best_orig is not None else 0

    # scatter corrections, last-t-wins per target prior
    last_t = {}
    for t in range(T):
        last_t[bpi_perm[t]] = t
    num = base_num
    den = base_den
    for q, t in last_t.items():
        f_old = 1.0 if bto[q] > ln_thresh else 0.0
        num -= salpha_p[q] * f_old * bto[q]
        num += salpha_p[q] * K * bpo[t]
        den += K - f_old
    loss = (-num + BETA * s_alpha) / den
    return np.float32(loss)


_NC_CACHE = {}


def run_cores(locs, params, truths, trace=False):
    locs = np.asarray(locs, dtype=np.float32)
    params = np.asarray(params, dtype=np.float32)
    truths = np.asarray(truths, dtype=np.float32)
    in_maps, meta = _host_prep(locs, params, truths)
    nblk = meta["nblk"]
    if nblk not in _NC_CACHE:
        _NC_CACHE[nblk] = build_nc(nblk)
    nc = _NC_CACHE[nblk]
    out = run_bass_kernel_spmd(nc, in_maps, list(range(NCORES)), trace=trace)
    return out, meta


def kernel(locs, params, truths):
    out, meta = run_cores(locs, params, truths, trace=False)
    return _combine(out.results, meta)


if __name__ == "__main__":
    rng = np.random.default_rng(0)
    locs = rng.random((P, 2), dtype=np.float32)
    params = np.concatenate(
        [rng.random((P, 2), dtype=np.float32) * 0.2 + 0.02,
         rng.standard_normal((P, 1), dtype=np.float32)], axis=1)
    t_c = rng.random((T, 2), dtype=np.float32)
    t_w = rng.random((T, 2), dtype=np.float32) * 0.3 + 0.1
    truths = np.concatenate([t_c - t_w / 2, t_c + t_w / 2], axis=1).astype(np.float32)
    truths[0] = [0.0, 0.0, 1.0, 1.0]
    print(kernel(locs, params, truths))


# revision 11
# speedup vs baseline: 1.1759x; 1.1759x over previous
"""AdaptivePriorBoxesLoss on 8 Trainium2 NeuronCores (Bass/Tile).

Sparse cell-bucketed formulation. Host quantile-bins the P=262144 priors
into 8 y-bands (one per core) x 16 x-cells of exactly 2048 priors. For
each cell only the truths whose boxes can overlap the cell's prior hull
are evaluated (~30 of 128). The (cell, truth) work pairs are packed into
128 partition-row "slots" x NBLK blocks; each row serves one fixed cell
(so its 2048-prior data is loaded once) and visits NBLK of its cell's
truths, one per block, delivered as per-partition scalars.

Per block the device computes, on [128, 2048] bf16 tiles:
    t1=min(px2,tx2)  t2=max(px1,tx1)  w=t1-t2   wr=relu(w)     (x chain)
    u1=min(py2,ty2)  u2=max(py1,ty1)  h=u1-u2   hr=relu(h)     (y chain)
    inter=wr*hr  den0=pa-inter
    lnI=Ln(inter)  lnD=Ln(den0+ta)          (scalar engine, f32)
    iou=lnI-lnD (+ fused per-row max -> MAXC[:,b])
    BTOP=max(BTOP, iou)                      (running best-truth-overlap)
    cand=(iou>=maxc*(1+2^-7)) * (idx-BIG); CID[:,b]=min(cand)  (argmax)

Host combines: per-cell max of BTOP slot rows -> ln(bto) per prior, the
filter/sum reductions, and the <=128 best-prior scatter corrections,
exactly mirroring the reference semantics.
"""

import os
import sys
from contextlib import ExitStack

for _p in ("/opt/trn_rl_repo", os.path.expanduser("~/.axon_site/_ro/trn_rl_repo")):
    if os.path.isdir(_p) and _p not in sys.path:
        sys.path.insert(0, _p)

import numpy as np
import ml_dtypes

import concourse.bass as bass
import concourse.bacc as bacc
import concourse.mybir as mybir
from concourse import tile
from concourse.bass_utils import run_bass_kernel_spmd
from concourse import dve_ops, dve_spec
from concourse.dve_spec import (
    Spec, Src0, Src1, C0, C1, C2, Zero, Idx, relu, minn, maxx, select, lower,
)
from concourse.dve_uop import DveOpSpec

BF16NP = ml_dtypes.bfloat16

P = 262144
T = 128
NCORES = 8
NCELL = 16
CPC = 2048
PPC = NCELL * CPC          # priors per core = 32768
BIG = 1048576.0
BETA = 1.0
K = 2.5
IOU_THRESH = 0.4

F32 = mybir.dt.float32
BF16 = mybir.dt.bfloat16
ALU = mybir.AluOpType
ACTF = mybir.ActivationFunctionType

NEG = -3.0e38
EPS_MUL = 1.0 + 2.0 ** -7   # widen argmax match by one bf16 ulp


def _register_dve_op(name, spec, subdim=False):
    """Register a custom DVE op at runtime (self-contained kernel.py)."""
    for op in dve_ops.OPS:
        if op.name == name:
            return op
    row = dve_ops._CUSTOM_DVE_ROW_BASE + len(dve_ops.OPS)
    assert row < 0x20, "custom-DVE opcode rows exhausted"
    dve_ops._SUB_OPCODE_FOR_NAME[name] = row
    shas = {}
    for ver in ("v3", "v4"):
        s = DveOpSpec(
            name=name, opcode=row, uops=lower(spec, ver=ver),
            rd1_en=dve_spec._has_src1(spec),
        )
        shas[ver] = s.sha(ver)
    op = dve_ops.DveOp(name, spec, subdim, uops_sha=shas)
    dve_ops.OPS.append(op)
    dve_ops.CUSTOM_DVE_SPECS[name] = spec
    return op


def _np_spanw(in0, in1, s0, s1, imm2):
    return np.maximum(
        np.minimum(in0.astype(np.float32), s0)
        - np.maximum(in1.astype(np.float32), s1), 0.0)


def _np_submax(in0, in1, s0, s1, imm2):
    b = in0.astype(np.float32) - in1.astype(np.float32)
    bm = np.where(np.isnan(b), -np.inf, b)
    acc = np.maximum(bm.max(axis=-1, keepdims=True).reshape(b.shape[0], -1)
                     .max(axis=-1, keepdims=True), s1)
    return b, acc


def _np_selmin(in0, in1, s0, s1, imm2):
    idx = np.arange(in0.shape[-1], dtype=np.float32)[None, :]
    b = np.where(in0.astype(np.float32) >= s0, idx, imm2).astype(np.float32)
    b = b.reshape(in0.shape[0], -1)
    acc = np.minimum(b.min(axis=-1, keepdims=True), s1)
    return b, acc


# wr = relu(min(hi, t_hi) - max(lo, t_lo)) — the clipped 1-D span
SPANW_ANT = _register_dve_op(
    "SPANW_ANT",
    Spec(body=relu(minn(Src0, C0) - maxx(Src1, C1)), reference=_np_spanw),
)
# out = in0 - in1; accum_out = max(out) (seeded from C1)
SUBMAX_ANT = _register_dve_op(
    "SUBMAX_ANT",
    Spec(body=Src0 - Src1, accum=maxx, accum_init=C1, reference=_np_submax),
)
# out = (in0 >= c0) ? elem_idx : imm2; accum_out = min(out) (seeded from C1)
SELMIN_ANT = _register_dve_op(
    "SELMIN_ANT",
    Spec(body=select(Src0 >= C0, Idx, C2), accum=minn,
         accum_init=C1, reference=_np_selmin),
)


def build_nc(nblk):
    nc = bacc.Bacc()

    pd_e = nc.declare_dram_parameter("pd", [128, 5 * CPC], BF16, isOutput=False)
    ts_e = nc.declare_dram_parameter("tscal", [128, 8 * nblk], F32, isOutput=False)
    btop_o = nc.declare_dram_parameter("btop_out", [128, CPC], BF16, isOutput=True)
    maxc_o = nc.declare_dram_parameter("maxc_out", [128, nblk], F32, isOutput=True)
    cid_o = nc.declare_dram_parameter("cid_out", [128, nblk], F32, isOutput=True)

    with ExitStack() as es:
        tc = es.enter_context(tile.TileContext(nc))
        cpool = es.enter_context(tc.tile_pool(name="const", bufs=1))
        wpool = es.enter_context(tc.tile_pool(name="work", bufs=3))

        TSCAL = cpool.tile([128, 8 * nblk], F32, tag="TSCAL")
        nc.sync.dma_start(out=TSCAL[:], in_=ts_e[:])

        def pdarr(i, tag):
            t_ = cpool.tile([128, CPC], BF16, tag=tag)
            nc.sync.dma_start(out=t_[:], in_=pd_e[:, i * CPC:(i + 1) * CPC])
            return t_[:]

        PX1 = pdarr(0, "PX1")
        PX2 = pdarr(1, "PX2")
        PY1 = pdarr(2, "PY1")
        PY2 = pdarr(3, "PY2")
        PA = pdarr(4, "PA")

        BTOP = cpool.tile([128, CPC], BF16, tag="BTOP")
        nc.vector.memset(BTOP[:], float("-inf"))
        MAXC = cpool.tile([128, nblk], F32, tag="MAXC")
        CID = cpool.tile([128, nblk], F32, tag="CID")

        for b in range(nblk):
            def sc(j):
                return TSCAL[:, 8 * b + j:8 * b + j + 1]

            wr = wpool.tile([128, CPC], BF16, tag="wr")
            nc.vector._custom_dve(
                SPANW_ANT, out=wr[:], in0=PX2, in1=PX1, s0=sc(0), s1=sc(1)
            )
            hr = wpool.tile([128, CPC], BF16, tag="hr")
            nc.vector._custom_dve(
                SPANW_ANT, out=hr[:], in0=PY2, in1=PY1, s0=sc(2), s1=sc(3)
            )

            inter = wpool.tile([128, CPC], BF16, tag="inter")
            nc.vector.tensor_tensor(inter[:], wr[:], hr[:], ALU.mult)
            den0 = wpool.tile([128, CPC], BF16, tag="den0")
            nc.vector.tensor_tensor(den0[:], PA, inter[:], ALU.subtract)

            lnI = wpool.tile([128, CPC], F32, tag="lnI")
            nc.scalar.activation(lnI[:], inter[:], ACTF.Ln)
            lnD = wpool.tile([128, CPC], F32, tag="lnD")
            nc.scalar.activation(lnD[:], den0[:], ACTF.Ln, bias=sc(4))

            iou = wpool.tile([128, CPC], BF16, tag="iou")
            nc.vector._custom_dve(
                SUBMAX_ANT, out=iou[:], in0=lnI[:], in1=lnD[:],
                s1=NEG, accum_out=MAXC[:, b:b + 1],
            )

            nc.vector.tensor_tensor(BTOP[:], BTOP[:], iou[:], ALU.max)

            mce = wpool.tile([128, 1], F32, tag="mce")
            nc.vector.tensor_scalar_mul(mce[:], MAXC[:, b:b + 1], EPS_MUL)
            cand = wpool.tile([128, CPC], F32, tag="cand")
            nc.vector._custom_dve(
                SELMIN_ANT, out=cand[:], in0=iou[:],
                s0=mce[:], s1=3.0e38, imm2=3.0e38,
                accum_out=CID[:, b:b + 1],
            )

        nc.sync.dma_start(out=btop_o[:], in_=BTOP[:])
        nc.sync.dma_start(out=maxc_o[:], in_=MAXC[:])
        nc.sync.dma_start(out=cid_o[:], in_=CID[:])

    nc.finalize()
    return nc


def _host_prep(locs, params, truths):
    """Bucket priors, build per-core slot schedules and device inputs."""
    px = locs[:, 0]
    py = locs[:, 1]
    pw = params[:, 0]
    ph = params[:, 1]
    px1 = px - pw / 2
    px2 = px + pw / 2
    py1 = py - ph / 2
    py2 = py + ph / 2
    pa = pw * ph
    salpha = 1.0 / (1.0 + np.exp(-params[:, 2].astype(np.float64)))

    order_y = np.argsort(py, kind="stable")
    flat_cells = np.empty((NCORES * NCELL, CPC), dtype=np.int64)
    for c in range(NCORES):
        band = order_y[c * PPC:(c + 1) * PPC]
        band = band[np.argsort(px[band], kind="stable")]
        flat_cells[c * NCELL:(c + 1) * NCELL] = band.reshape(NCELL, CPC)

    tx1, ty1, tx2, ty2 = truths[:, 0], truths[:, 1], truths[:, 2], truths[:, 3]
    ta = (tx2 - tx1) * (ty2 - ty1)

    # per-cell truth lists from exact prior-box hulls
    ncc = NCORES * NCELL
    flat_lists = []
    for cc in range(ncc):
        pp = flat_cells[cc]
        hx1 = px1[pp].min(); hx2 = px2[pp].max()
        hy1 = py1[pp].min(); hy2 = py2[pp].max()
        hit = (tx1 <= hx2) & (tx2 >= hx1) & (ty1 <= hy2) & (ty2 >= hy1)
        flat_lists.append(np.nonzero(hit)[0])

    # pick smallest NBLK for which cells can be bin-packed into cores with
    # <=128 row-slots each (LPT greedy), then apply that assignment
    nblk = None
    for cand in range(1, 17):
        slots = np.array([-(-len(l) // cand) for l in flat_lists])
        order = np.argsort(-slots, kind="stable")
        loads = np.zeros(NCORES, dtype=np.int64)
        counts = np.zeros(NCORES, dtype=np.int64)
        assign = np.full(ncc, -1, dtype=np.int64)
        for cc in order:
            feas = np.nonzero(counts < NCELL)[0]
            tgt = feas[np.argmin(loads[feas])]
            assign[cc] = tgt
            loads[tgt] += slots[cc]
            counts[tgt] += 1
        if loads.max() <= 128:
            nblk = cand
            break
    assert nblk is not None

    perm = np.empty(P, dtype=np.int64)
    pp_all = np.empty((NCORES, NCELL, CPC), dtype=np.int64)
    lists = [[] for _ in range(NCORES)]
    fill = np.zeros(NCORES, dtype=np.int64)
    for cc in range(ncc):
        c = assign[cc]
        g = fill[c]
        fill[c] += 1
        pp_all[c, g] = flat_cells[cc]
        lists[c].append(flat_lists[cc])
        perm[c * PPC + g * CPC:(c * PPC + (g + 1) * CPC)] = flat_cells[cc]

    # slot assignment
    rowcell = np.full((NCORES, 128), -1, dtype=np.int64)
    rowslot = np.zeros((NCORES, 128), dtype=np.int64)
    rowbase = np.zeros((NCORES, NCELL), dtype=np.int64)
    for c in range(NCORES):
        r = 0
        for g in range(NCELL):
            rowbase[c, g] = r
            ns = -(-len(lists[c][g]) // nblk)
            for k in range(ns):
                rowcell[c, r] = g
                rowslot[c, r] = k
                r += 1

    in_maps = []
    for c in range(NCORES):
        pd = np.zeros((128, 5 * CPC), dtype=BF16NP)
        tscal = np.zeros((128, 8 * nblk), dtype=np.float32)
        for r in range(128):
            g = rowcell[c, r]
            if g < 0:
                tscal[r, 0::8] = -9.99
                tscal[r, 1::8] = -10.0
                tscal[r, 2::8] = -9.99
                tscal[r, 3::8] = -10.0
                tscal[r, 4::8] = 1.0
                continue
            pp = pp_all[c, g]
            pd[r, 0 * CPC:1 * CPC] = px1[pp].astype(BF16NP)
            pd[r, 1 * CPC:2 * CPC] = px2[pp].astype(BF16NP)
            pd[r, 2 * CPC:3 * CPC] = py1[pp].astype(BF16NP)
            pd[r, 3 * CPC:4 * CPC] = py2[pp].astype(BF16NP)
            pd[r, 4 * CPC:5 * CPC] = pa[pp].astype(BF16NP)
            lst = lists[c][g]
            k = rowslot[c, r]
            for b in range(nblk):
                pos = k * nblk + b
                if pos < len(lst):
                    t = lst[pos]
                    tscal[r, 8 * b + 0] = tx2[t]
                    tscal[r, 8 * b + 1] = tx1[t]
                    tscal[r, 8 * b + 2] = ty2[t]
                    tscal[r, 8 * b + 3] = ty1[t]
                    tscal[r, 8 * b + 4] = ta[t]
                else:
                    tscal[r, 8 * b + 0] = -9.99
                    tscal[r, 8 * b + 1] = -10.0
                    tscal[r, 8 * b + 2] = -9.99
                    tscal[r, 8 * b + 3] = -10.0
                    tscal[r, 8 * b + 4] = 1.0
        in_maps.append({"pd": pd, "tscal": tscal})

    meta = dict(
        perm=perm, lists=lists, nblk=nblk, rowcell=rowcell,
        rowbase=rowbase, salpha=salpha, pp_all=pp_all,
    )
    return in_maps, meta


def _combine(results, meta):
    perm = meta["perm"]
    lists = meta["lists"]
    nblk = meta["nblk"]
    rowcell = meta["rowcell"]
    rowbase = meta["rowbase"]
    salpha = meta["salpha"]

    ln_thresh = np.log(IOU_THRESH)

    bto = np.full(P, -np.inf, dtype=np.float64)   # permuted order, ln-domain
    maxc = []
    cid = []
    for c in range(NCORES):
        btop = np.asarray(results[c]["btop_out"]).astype(np.float32)
        maxc.append(np.asarray(results[c]["maxc_out"]))
        cid.append(np.asarray(results[c]["cid_out"]))
        for g in range(NCELL):
            rows = np.nonzero(rowcell[c] == g)[0]
            m = btop[rows[0]]
            for r in rows[1:]:
                m = np.maximum(m, btop[r])
            bto[c * PPC + g * CPC:(c * PPC + (g + 1) * CPC)] = m

    salpha_p = salpha[perm]
    F = bto > ln_thresh
    s_alpha = salpha.sum()
    base_num = (salpha_p[F] * bto[F]).sum()
    base_den = float(F.sum())

    # per-truth winner (bpo in ln domain, bpi as permuted global index)
    bpo = np.full(T, -np.inf)
    bpi_perm = np.zeros(T, dtype=np.int64)
    bpi_orig = np.zeros(T, dtype=np.int64)
    for t in range(T):
        best = -np.inf
        best_orig = None
        best_perm = 0
        for c in range(NCORES):
            for g in range(NCELL):
                pos_arr = np.nonzero(lists[c][g] == t)[0]
                if not len(pos_arr):
                    continue
                pos = int(pos_arr[0])
                k, b = divmod(pos, nblk)
                r = rowbase[c, g] + k
                m = float(maxc[c][r, b])
                if m <= NEG:
                    continue
                idx = cid[c][r, b]
                if not (0 <= idx < CPC):
                    idx = 0.0
                gp = c * PPC + g * CPC + int(idx)
                go = int(perm[gp])
                if m > best or (m == best and go < best_orig):
                    best = m
                    best_orig = go
                    best_perm = gp
        bpo[t] = best
        bpi_perm[t] = best_perm
        bpi_orig[t] = best_orig if best_orig is not None else 0

    # scatter corrections, last-t-wins per target prior
    last_t = {}
    for t in range(T):
        last_t[bpi_perm[t]] = t
    num = base_num
    den = base_den
    for q, t in last_t.items():
        f_old = 1.0 if bto[q] > ln_thresh else 0.0
        num -= salpha_p[q] * f_old * bto[q]
        num += salpha_p[q] * K * bpo[t]
        den += K - f_old
    loss = (-num + BETA * s_alpha) / den
    return np.float32(loss)


_NC_CACHE = {}


def run_cores(locs, params, truths, trace=False):
    locs = np.asarray(locs, dtype=np.float32)
    params = np.asarray(params, dtype=np.float32)
    truths = np.asarray(truths, dtype=np.float32)
    in_maps, meta = _host_prep(locs, params, truths)
    nblk = meta["nblk"]
    if nblk not in _NC_CACHE:
        _NC_CACHE[nblk] = build_nc(nblk)
    nc = _NC_CACHE[nblk]
    out = run_bass_kernel_spmd(nc, in_maps, list(range(NCORES)), trace=trace)
    return out, meta


def kernel(locs, params, truths):
    out, meta = run_cores(locs, params, truths, trace=False)
    return _combine(out.results, meta)


if __name__ == "__main__":
    rng = np.random.default_rng(0)
    locs = rng.random((P, 2), dtype=np.float32)
    params = np.concatenate(
        [rng.random((P, 2), dtype=np.float32) * 0.2 + 0.02,
         rng.standard_normal((P, 1), dtype=np.float32)], axis=1)
    t_c = rng.random((T, 2), dtype=np.float32)
    t_w = rng.random((T, 2), dtype=np.float32) * 0.3 + 0.1
    truths = np.concatenate([t_c - t_w / 2, t_c + t_w / 2], axis=1).astype(np.float32)
    truths[0] = [0.0, 0.0, 1.0, 1.0]
    print(kernel(locs, params, truths))
